# revision 25
# baseline (speedup 1.0000x reference)
"""DiT block kernel for Trainium2, data-parallel over batch across 8 NeuronCores.

Problem: nn_DiTBlock — B=8, S=1024, E=512, H=8 (head_dim = E = 512).
Sharding: batch element b -> core b. Each core runs the full DiT block on its
(S, E) slice with replicated weights; no collectives.

Per-core dataflow (activations kept transposed so the contraction dim sits on
partitions; float32r matmuls = full-rate PE with ~1.5e-4 relative rounding):
  AdaLN matvecs (PE) -> LN1 stats (DVE bn_stats) -> PE-transpose with fused
  modulate (ACT per-partition scale/bias) -> y^T
  -> software-pipelined head loop: QK(h) emitted between scores(h-1) and
     AV(h-1) so exp/tree latency hides under PE work
  -> residual -> LN2 (same transpose-modulate trick) -> FFN (relu in ACT)
  -> out = y + (h @ f2w + f2b) * alpha2.
"""
import sys
import numpy as np

sys.path.insert(0, '/opt/trn_rl_repo')

B, S, E, H = 8, 1024, 512, 8
HE = H * E          # 4096
FF = 4 * E          # 2048
EPS = 1e-5
SCALE = 1.0 / 32.0  # 1/sqrt(S)
N_CORES = 8

TRACE = False       # set by test harness to capture an NTFF profile
TRACE_DIR = None

_CACHE = {}


def _build():
    from contextlib import ExitStack
    import concourse.bass as bass
    import concourse.tile as tile
    from concourse import bacc, mybir
    f32 = mybir.dt.float32
    f32r = mybir.dt.float32r
    bf16 = mybir.dt.bfloat16
    AF = mybir.ActivationFunctionType
    ALU = mybir.AluOpType

    nc = bacc.Bacc("TRN2", target_bir_lowering=False, debug=False,
                   num_devices=N_CORES)

    # ---- DRAM parameters --------------------------------------------------
    x_d = nc.dram_tensor("x", [S, E], f32, kind="ExternalInput").ap()
    cond_d = nc.dram_tensor("cond", [E, 1], f32, kind="ExternalInput").ap()

    adaln_w = {}
    adaln_b = {}
    for nm in ["g1", "be1", "a1", "g2", "be2", "a2"]:
        adaln_w[nm] = nc.dram_tensor(f"{nm}w", [E, E], f32,
                                     kind="ExternalInput").ap()
        adaln_b[nm] = nc.dram_tensor(f"{nm}b", [1, E], f32,
                                     kind="ExternalInput").ap()
    ln1g_d = nc.dram_tensor("ln1g", [1, E], f32, kind="ExternalInput").ap()
    ln1b_d = nc.dram_tensor("ln1b", [1, E], f32, kind="ExternalInput").ap()
    ln2g_d = nc.dram_tensor("ln2g", [1, E], f32, kind="ExternalInput").ap()
    ln2b_d = nc.dram_tensor("ln2b", [1, E], f32, kind="ExternalInput").ap()
    wq_d = nc.dram_tensor("wq", [E, HE], f32r, kind="ExternalInput").ap()
    wk_d = nc.dram_tensor("wk", [E, HE], f32r, kind="ExternalInput").ap()
    wv_d = nc.dram_tensor("wv", [E, HE], f32r, kind="ExternalInput").ap()
    bq_d = nc.dram_tensor("bq", [1, HE], f32, kind="ExternalInput").ap()
    bk_d = nc.dram_tensor("bk", [1, HE], f32, kind="ExternalInput").ap()
    bv_d = nc.dram_tensor("bv", [1, HE], f32, kind="ExternalInput").ap()
    lvw_d = nc.dram_tensor("lvw", [HE, E], f32r, kind="ExternalInput").ap()
    lvb_d = nc.dram_tensor("lvb", [1, E], f32, kind="ExternalInput").ap()
    f1w_d = nc.dram_tensor("f1w", [E, FF], f32r, kind="ExternalInput").ap()
    f1b_d = nc.dram_tensor("f1b", [1, FF], f32, kind="ExternalInput").ap()
    f2w_d = nc.dram_tensor("f2w", [FF, E], f32r, kind="ExternalInput").ap()
    f2b_d = nc.dram_tensor("f2b", [1, E], f32, kind="ExternalInput").ap()
    ident_d = nc.dram_tensor("ident", [128, 128], f32r,
                             kind="ExternalInput").ap()
    out_d = nc.dram_tensor("out", [S, E], f32, kind="ExternalOutput").ap()

    with tile.TileContext(nc) as tc, ExitStack() as ctx:
        const = ctx.enter_context(tc.tile_pool(name="const", bufs=1))
        work = ctx.enter_context(tc.tile_pool(name="work", bufs=3))
        psum_mm = ctx.enter_context(
            tc.tile_pool(name="psum_mm", bufs=4, space="PSUM"))
        psum_tp = ctx.enter_context(
            tc.tile_pool(name="psum_tp", bufs=2, space="PSUM"))
        psum_row = ctx.enter_context(
            tc.tile_pool(name="psum_row", bufs=2, space="PSUM"))

        # ---- constants ----
        ident = const.tile([128, 128], f32r)
        nc.sync.dma_start(ident, ident_d)
        eps_t = const.tile([128, 1], f32)
        nc.vector.memset(eps_t, EPS)
        ones_col = const.tile([128, 1], f32)
        nc.vector.memset(ones_col, 1.0)

        cond_cols = const.tile([128, 4], f32)
        nc.sync.dma_start(cond_cols, cond_d.rearrange("(c p) o -> p (c o)", p=128))

        adp = ctx.enter_context(tc.tile_pool(name="adp", bufs=2))

        def adaln_cols(nm):
            """(cond @ W + b) laid out as [128, 4] e-columns (PE matvec)."""
            pcol = psum_tp.tile([128, 4], f32, name=f"pcol_{nm}", tag="ptp")
            for kc in range(4):
                adw = adp.tile([128, E], f32, name=f"adw_{nm}_{kc}", tag="adw")
                nc.sync.dma_start(adw, adaln_w[nm][kc * 128:(kc + 1) * 128, :])
                for ec in range(4):
                    nc.tensor.matmul(
                        pcol[:, ec:ec + 1],
                        adw[:, ec * 128:(ec + 1) * 128],
                        cond_cols[:, kc:kc + 1],
                        start=(kc == 0 and ec == 0),
                        stop=(kc == 3 and ec == 3))
            bcol = adp.tile([128, 4], f32, name=f"bcol_{nm}", tag="bcol")
            nc.sync.dma_start(
                bcol, adaln_b[nm].rearrange("o (c p) -> (o p) c", p=128))
            mcol = const.tile([128, 4], f32, name=f"mcol_{nm}")
            nc.vector.tensor_add(mcol, pcol, bcol)
            return mcol

        def adaln_rep(nm):
            """(cond @ W + b) replicated to [128, E] (row matvec + bcast)."""
            prow = psum_row.tile([1, E], f32, name=f"prow_{nm}", tag="prow")
            for kc in range(4):
                adw = adp.tile([128, E], f32, name=f"adw_{nm}_{kc}", tag="adw")
                nc.sync.dma_start(adw, adaln_w[nm][kc * 128:(kc + 1) * 128, :])
                nc.tensor.matmul(prow, cond_cols[:, kc:kc + 1], adw,
                                 start=(kc == 0), stop=(kc == 3))
            brow = adp.tile([1, E], f32, name=f"brow_{nm}", tag="brow")
            nc.sync.dma_start(brow, adaln_b[nm])
            arow = adp.tile([1, E], f32, name=f"arow_{nm}", tag="arow")
            nc.vector.tensor_add(arow, prow, brow)
            arep = const.tile([128, E], f32, name=f"arep_{nm}")
            nc.gpsimd.partition_broadcast(arep, arow)
            return arep

        def scale_shift(li, gcol, bcol, lng_d, lnb_d):
            """sc = ln_g*(1+gamma), bi = ln_b*(1+gamma)+beta, as [128,4] cols."""
            gp = const.tile([128, 4], f32, name=f"gp_{li}")
            nc.vector.tensor_scalar_add(gp, gcol, 1.0)
            lgc = adp.tile([128, 4], f32, name=f"lgc_{li}", tag="lgc")
            nc.sync.dma_start(lgc, lng_d.rearrange("o (c p) -> (o p) c", p=128))
            lbc = adp.tile([128, 4], f32, name=f"lbc_{li}", tag="lbc")
            nc.sync.dma_start(lbc, lnb_d.rearrange("o (c p) -> (o p) c", p=128))
            sc = const.tile([128, 4], f32, name=f"sc_{li}")
            nc.vector.tensor_mul(sc, lgc, gp)
            bi = const.tile([128, 4], f32, name=f"bi_{li}")
            nc.vector.tensor_mul(bi, lbc, gp)
            nc.vector.tensor_add(bi, bi, bcol)
            return sc, bi

        # Only g1/be1 gate the LN1->y^T critical path; defer the rest.
        sc1, bi1 = scale_shift(0, adaln_cols("g1"), adaln_cols("be1"),
                               ln1g_d, ln1b_d)

        # ---- persistent activation tiles ----
        yT = [const.tile([128, S], bf16, name=f"yT{c}") for c in range(4)]
        y2acc = [const.tile([128, E], f32, name=f"y2acc{t}") for t in range(8)]

        def layernorm_transpose(src_tiles, scol, bcol, dst_T, tagp):
            """LN over free dim of [128, E] tiles; PE-transpose each 128-block;
            fused (scale, bias) per-partition modulation on the PSUM->SBUF copy."""
            for t in range(8):
                if src_tiles is None:
                    x_t = work.tile([128, E], f32, name=f"xt_{tagp}", tag="wbig")
                    nc.sync.dma_start(x_t, x_d[t * 128:(t + 1) * 128, :])
                else:
                    x_t = src_tiles[t]
                st = work.tile([128, 6], f32, name=f"st_{tagp}", tag=f"st_{tagp}")
                nc.vector.bn_stats(st, x_t)
                mv = work.tile([128, 2], f32, name=f"mv_{tagp}", tag=f"mv_{tagp}")
                nc.vector.bn_aggr(mv, st)
                rs = work.tile([128, 1], f32, name=f"rs_{tagp}", tag=f"rs_{tagp}")
                nc.scalar.activation(rs, mv[:, 1:2], AF.Sqrt, bias=eps_t, scale=1.0)
                nc.vector.reciprocal(rs, rs)
                xn = work.tile([128, E], f32r, name=f"xn_{tagp}", tag="wxn")
                nc.vector.tensor_scalar(xn, x_t, scalar1=mv[:, 0:1], scalar2=rs,
                                        op0=ALU.subtract, op1=ALU.mult)
                for ec in range(4):
                    tp = psum_tp.tile([128, 128], f32r, name=f"tp_{tagp}",
                                      tag="ptp")
                    nc.tensor.transpose(tp, xn[:, ec * 128:(ec + 1) * 128], ident)
                    nc.scalar.activation(
                        dst_T[ec][:, t * 128:(t + 1) * 128], tp, AF.Identity,
                        bias=bcol[:, ec:ec + 1], scale=scol[:, ec:ec + 1])

        # PE warmup burst (~20 matmuls, ~4.5us) so the HAM clock-gate opens
        # before the transpose/QKV stream begins.
        warm_ps = psum_mm.tile([128, 512], f32, name="warm", tag="pmm")
        for i in range(20):
            nc.tensor.matmul(warm_ps[:, 0:128], ident, ident,
                             start=(i == 0), stop=(i == 19))

        # ---- Phase 1: LN1 -> y^T ----
        layernorm_transpose(None, sc1, bi1, yT, "ln1")

        # Deferred AdaLN is emitted from inside the head loop (see below) so
        # its weight DMAs stay off the phase-0/head-0 DMA critical path.
        MOD = {}

        def emit_adaln_a1_seed():
            MOD["A1"] = adaln_rep("a1")
            # y = x + (o@lvw + lvb)*a1 accumulated head by head with a1
            # folded into lvw; seed the accumulator with x + lvb*a1.
            LVBA = const.tile([128, E], f32)
            nc.sync.dma_start(LVBA, lvb_d.broadcast_to([128, E]))
            nc.vector.tensor_mul(LVBA, LVBA, MOD["A1"])
            for t in range(8):
                x_t3 = work.tile([128, E], f32, name="xt3", tag="wbig")
                nc.sync.dma_start(x_t3, x_d[t * 128:(t + 1) * 128, :])
                nc.vector.tensor_add(y2acc[t], x_t3, LVBA)

        def emit_adaln_ln2():
            MOD["sc2"], MOD["bi2"] = scale_shift(
                1, adaln_cols("g2"), adaln_cols("be2"), ln2g_d, ln2b_d)
            MOD["A2"] = adaln_rep("a2")

        # LN2 emitted per tile from inside the last head's lv loop, so its
        # DVE chain hides under the remaining lv matmuls.
        zT = [const.tile([128, S], f32r, name=f"zT{c}", tag=f"yT{c}")
              for c in range(4)]

        def ln2_tile(t):
            st2 = work.tile([128, 6], f32, name="st_ln2", tag="st_ln2")
            nc.vector.bn_stats(st2, y2acc[t])
            mv2 = work.tile([128, 2], f32, name="mv_ln2", tag="mv_ln2")
            nc.vector.bn_aggr(mv2, st2)
            rs2 = work.tile([128, 1], f32, name="rs_ln2", tag="rs_ln2")
            nc.scalar.activation(rs2, mv2[:, 1:2], AF.Sqrt, bias=eps_t, scale=1.0)
            nc.vector.reciprocal(rs2, rs2)
            xn2 = work.tile([128, E], f32r, name="xn_ln2", tag="wxn")
            nc.vector.tensor_scalar(xn2, y2acc[t], scalar1=mv2[:, 0:1],
                                    scalar2=rs2, op0=ALU.subtract, op1=ALU.mult)
            for ec in range(4):
                tp2 = psum_tp.tile([128, 128], f32r, name="tp_ln2", tag="ptp")
                nc.tensor.transpose(tp2, xn2[:, ec * 128:(ec + 1) * 128], ident)
                nc.scalar.activation(
                    zT[ec][:, t * 128:(t + 1) * 128], tp2, AF.Identity,
                    bias=MOD["bi2"][:, ec:ec + 1], scale=MOD["sc2"][:, ec:ec + 1])

        LN2_CB = [ln2_tile]

        # ---- Phase 2: attention heads (software-pipelined) ----
        with tc.tile_pool(name="hp", bufs=1) as hp:

            def head_qk(h):
                """Load wq/wk slices, compute Q^T, K^T for head h."""
                hof = h * E
                wq_t = [hp.tile([128, E], bf16, name=f"wq{kc}", tag=f"wq{kc}")
                        for kc in range(4)]
                wk_t = [hp.tile([128, E], bf16, name=f"wk{kc}", tag=f"wk{kc}")
                        for kc in range(4)]
                for kc in range(4):
                    nc.gpsimd.dma_start(
                        wq_t[kc], wq_d[kc * 128:(kc + 1) * 128, hof:hof + E])
                    nc.gpsimd.dma_start(
                        wk_t[kc], wk_d[kc * 128:(kc + 1) * 128, hof:hof + E])
                bqc = hp.tile([128, 4], f32, tag="bqc")
                nc.sync.dma_start(
                    bqc, bq_d[0:1, hof:hof + E].rearrange("o (c p) -> (o p) c",
                                                          p=128))
                bkc = hp.tile([128, 4], f32, tag="bkc")
                nc.sync.dma_start(
                    bkc, bk_d[0:1, hof:hof + E].rearrange("o (c p) -> (o p) c",
                                                          p=128))
                QT = [hp.tile([128, S], bf16, name=f"QT{mc}", tag=f"QT{mc}")
                      for mc in range(4)]
                KT = [hp.tile([128, S], bf16, name=f"KT{mc}", tag=f"KT{mc}")
                      for mc in range(4)]
                for mc in range(4):
                    for sh in range(2):
                        pq = psum_mm.tile([128, 512], f32, name="pq", tag="pmm")
                        for kc in range(4):
                            nc.tensor.matmul(
                                pq, wq_t[kc][:, mc * 128:(mc + 1) * 128],
                                yT[kc][:, sh * 512:(sh + 1) * 512],
                                start=(kc == 0), stop=(kc == 3))
                        nc.vector.tensor_scalar_add(
                            QT[mc][:, sh * 512:(sh + 1) * 512], pq,
                            bqc[:, mc:mc + 1])
                        pk = psum_mm.tile([128, 512], f32, name="pk", tag="pmm")
                        for kc in range(4):
                            nc.tensor.matmul(
                                pk, wk_t[kc][:, mc * 128:(mc + 1) * 128],
                                yT[kc][:, sh * 512:(sh + 1) * 512],
                                start=(kc == 0), stop=(kc == 3))
                        nc.vector.tensor_scalar_add(
                            KT[mc][:, sh * 512:(sh + 1) * 512], pk,
                            bkc[:, mc:mc + 1])
                return QT, KT

            def head_v(h):
                """Load wv slice, compute V (natural layout) for head h."""
                hof = h * E
                wv_t = [hp.tile([128, E], bf16, name=f"wv{kc}", tag=f"wv{kc}")
                        for kc in range(4)]
                for kc in range(4):
                    nc.gpsimd.dma_start(
                        wv_t[kc], wv_d[kc * 128:(kc + 1) * 128, hof:hof + E])
                BVrep = hp.tile([128, E], f32, tag="bvrep")
                nc.sync.dma_start(
                    BVrep, bv_d[0:1, hof:hof + E].broadcast_to([128, E]))
                Vh = [hp.tile([128, E], bf16, name=f"V{tc_}", tag=f"V{tc_}")
                      for tc_ in range(8)]
                for tc_ in range(8):
                    pv = psum_mm.tile([128, 512], f32, name="pv", tag="pmm")
                    for kc in range(4):
                        nc.tensor.matmul(
                            pv, yT[kc][:, tc_ * 128:(tc_ + 1) * 128], wv_t[kc],
                            start=(kc == 0), stop=(kc == 3))
                    nc.vector.tensor_add(Vh[tc_], pv, BVrep)
                return Vh

            def head_scores(QT, KT):
                """scores^T + exp; incremental DVE tree for denominators."""
                Eh = [hp.tile([128, S], bf16, name=f"E{tc_}", tag=f"E{tc_}")
                      for tc_ in range(8)]
                esum = hp.tile([128, S], f32, tag="esum")
                for tc_ in range(8):
                    for sh in range(2):
                        ps = psum_mm.tile([128, 512], f32, name="ps", tag="pmm")
                        for ec in range(4):
                            nc.tensor.matmul(
                                ps, KT[ec][:, tc_ * 128:(tc_ + 1) * 128],
                                QT[ec][:, sh * 512:(sh + 1) * 512],
                                start=(ec == 0), stop=(ec == 3))
                        nc.scalar.activation(
                            Eh[tc_][:, sh * 512:(sh + 1) * 512], ps, AF.Exp,
                            scale=SCALE)
                    if tc_ == 1:
                        nc.vector.tensor_add(esum, Eh[0], Eh[1])
                    elif tc_ > 1:
                        nc.vector.tensor_add(esum, esum, Eh[tc_])
                return Eh, esum

            def head_sums(esum):
                """Softmax denominators -> replicated reciprocal rows.
                Copy PSUM rows out fast (frees the bank), broadcast, then
                take the reciprocal across all 128 lanes."""
                srow = hp.tile([1, S], f32, tag="srow")
                for sh in range(2):
                    psr = psum_row.tile([1, 512], f32, name="psr", tag="prow")
                    nc.tensor.matmul(psr, ones_col,
                                     esum[:, sh * 512:(sh + 1) * 512],
                                     start=True, stop=True)
                    nc.vector.tensor_copy(srow[0:1, sh * 512:(sh + 1) * 512], psr)
                Srep = hp.tile([128, S], f32, tag="srep")
                nc.gpsimd.partition_broadcast(Srep, srow)
                Rrep = hp.tile([128, S], f32, tag="rrep")
                nc.vector.reciprocal_approx_fast(Rrep, Srep)
                return Rrep

            def head_av(Vh, Eh, Rrep):
                """AV matmuls; 1/sum applied on the PSUM->SBUF move."""
                oT = [hp.tile([128, S], f32r, name=f"oT{ec}", tag=f"oT{ec}")
                      for ec in range(4)]
                for ec in range(4):
                    for sh in range(2):
                        po = psum_mm.tile([128, 512], f32, name="po", tag="pmm")
                        for tc_ in range(8):
                            nc.tensor.matmul(
                                po, Vh[tc_][:, ec * 128:(ec + 1) * 128],
                                Eh[tc_][:, sh * 512:(sh + 1) * 512],
                                start=(tc_ == 0), stop=(tc_ == 7))
                        nc.vector.tensor_mul(
                            oT[ec][:, sh * 512:(sh + 1) * 512], po,
                            Rrep[:, sh * 512:(sh + 1) * 512])
                return oT

            def head_lv(h, oT, ln2_cb=None):
                """lv partial accumulate (alpha1 pre-folded into lvw)."""
                hof = h * E
                lvw_t = [hp.tile([128, E], f32r, name=f"lvw{kc}", tag=f"lvw{kc}")
                         for kc in range(4)]
                for kc in range(4):
                    nc.sync.dma_start(
                        lvw_t[kc], lvw_d[hof + kc * 128:hof + (kc + 1) * 128, :])
                    nc.vector.tensor_mul(lvw_t[kc], lvw_t[kc], MOD["A1"])
                for t in range(8):
                    py = psum_mm.tile([128, 512], f32, name="py", tag="pmm")
                    for kc in range(4):
                        nc.tensor.matmul(
                            py, oT[kc][:, t * 128:(t + 1) * 128], lvw_t[kc],
                            start=(kc == 0), stop=(kc == 3))
                    nc.vector.tensor_add(y2acc[t], y2acc[t], py)
                    if ln2_cb is not None:
                        ln2_cb(t)

            # Pipelined loop. PE stream per iteration:
            #   QK(h) -> sums(h-1) -> AV(h-1) -> V(h) -> scores(h) -> lv(h-1)
            # so exp/tree/recip of h-1 all hide under dense PE work.
            prev = None
            prev_oT = None
            for h in range(H):
                QT, KT = head_qk(h)
                if prev is not None:
                    Rrep = head_sums(prev[3])
                    prev_oT = head_av(prev[1], prev[2], Rrep)
                Vh = head_v(h)
                Eh, esum = head_scores(QT, KT)
                if prev is not None:
                    head_lv(prev[0], prev_oT)
                if h == 0:
                    emit_adaln_a1_seed()
                elif h == 2:
                    emit_adaln_ln2()
                prev = (h, Vh, Eh, esum)
            Rrep = head_sums(prev[3])
            prev_oT = head_av(prev[1], prev[2], Rrep)
            head_lv(prev[0], prev_oT, ln2_cb=LN2_CB[0])

        # ---- Phase 5 prefetch: FFN weights (hidden under residual/LN2) ----
        fp = ctx.enter_context(tc.tile_pool(name="fp", bufs=1))
        f1w_t = [fp.tile([128, FF], f32r, name=f"f1w{kc}", tag=f"f1w{kc}")
                 for kc in range(4)]
        for kc in range(4):
            nc.sync.dma_start(f1w_t[kc], f1w_d[kc * 128:(kc + 1) * 128, :])
        f1bc = fp.tile([128, 16], f32, tag="f1bc")
        nc.sync.dma_start(f1bc, f1b_d.rearrange("o (c p) -> (o p) c", p=128))

        # keep the PE clock-gate open across the LN2/weight-DMA boundary
        for i in range(12):
            warm2 = psum_mm.tile([128, 512], f32, name="warm2", tag="pmm")
            nc.tensor.matmul(warm2[:, 0:128], ident, ident,
                             start=True, stop=True)

        # ---- Phase 5: FFN ----
        hT = [fp.tile([128, S], f32r, name=f"hT{hc}", tag=f"hT{hc}")
              for hc in range(16)]
        for sh in range(2):
            for hc in range(16):
                pf = psum_mm.tile([128, 512], f32, name="pf", tag="pmm")
                for kc in range(4):
                    nc.tensor.matmul(
                        pf, f1w_t[kc][:, hc * 128:(hc + 1) * 128],
                        zT[kc][:, sh * 512:(sh + 1) * 512],
                        start=(kc == 0), stop=(kc == 3))
                nc.scalar.activation(
                    hT[hc][:, sh * 512:(sh + 1) * 512], pf, AF.Relu,
                    bias=f1bc[:, hc:hc + 1], scale=1.0)
        f2w_t = [fp.tile([128, E], f32r, name=f"f2w{kc}", tag=f"f2w{kc}")
                 for kc in range(16)]
        for kc in range(16):
            nc.sync.dma_start(f2w_t[kc], f2w_d[kc * 128:(kc + 1) * 128, :])
        F2B_rep = fp.tile([128, E], f32, tag="f2brep")
        nc.sync.dma_start(F2B_rep, f2b_d.broadcast_to([128, E]))
        for t in range(8):
            pz = psum_mm.tile([128, 512], f32, name="pz", tag="pmm")
            for kc in range(16):
                nc.tensor.matmul(
                    pz, hT[kc][:, t * 128:(t + 1) * 128], f2w_t[kc],
                    start=(kc == 0), stop=(kc == 15))
            q1 = work.tile([128, E], f32, name="q1", tag="wbig")
            nc.vector.tensor_add(q1, pz, F2B_rep)
            nc.vector.tensor_mul(q1, q1, MOD["A2"])
            ot = work.tile([128, E], f32, name="ot", tag="wbig")
            nc.vector.tensor_add(ot, q1, y2acc[t])
            nc.sync.dma_start(out_d[t * 128:(t + 1) * 128, :], ot)

    nc.compile()
    return nc


def _get_program():
    if "nc" not in _CACHE:
        _CACHE["nc"] = _build()
    return _CACHE["nc"]


def kernel(**inputs) -> np.ndarray:
    from concourse.bass_utils import run_bass_kernel_spmd

    ins = {k: np.asarray(v, dtype=np.float32) for k, v in inputs.items()}
    nc = _get_program()

    in_maps = []
    for b in range(B):
        m = {
            "x": ins["x"][b],                       # (S, E)
            "cond": ins["cond"][b].reshape(E, 1),   # (E, 1)
            "ln1g": ins["ln1g"].reshape(1, E), "ln1b": ins["ln1b"].reshape(1, E),
            "ln2g": ins["ln2g"].reshape(1, E), "ln2b": ins["ln2b"].reshape(1, E),
            "wq": ins["wq"], "wk": ins["wk"], "wv": ins["wv"],
            "bq": ins["bq"].reshape(1, HE), "bk": ins["bk"].reshape(1, HE),
            "bv": ins["bv"].reshape(1, HE),
            "lvw": ins["lvw"], "lvb": ins["lvb"].reshape(1, E),
            "f1w": ins["f1w"], "f1b": ins["f1b"].reshape(1, FF),
            "f2w": ins["f2w"], "f2b": ins["f2b"].reshape(1, E),
            "ident": np.eye(128, dtype=np.float32),
        }
        for nm in ["g1", "be1", "a1", "g2", "be2", "a2"]:
            m[f"{nm}w"] = ins[f"{nm}w"]
            m[f"{nm}b"] = ins[f"{nm}b"].reshape(1, E)
        in_maps.append(m)

    res = run_bass_kernel_spmd(nc, in_maps, list(range(N_CORES)),
                               trace=TRACE, tmpdir=TRACE_DIR)
    _CACHE["last_result"] = res
    out = np.stack([res.results[b]["out"] for b in range(B)], axis=0)
    return out
